# revision 1
# baseline (speedup 1.0000x reference)
"""Bass/Trainium2 kernel for nn_EuclideanGraphEncoder (GCN message passing).

Strategy: data-parallel over the batch (4 graphs per core, 8 cores),
weights replicated, no collectives.

Design (measured ~56us vs 84us fp16 baseline):
  - fp8 DoubleRow aggregation: the adjacency ships as fp8e4 (x16 scale)
    in a pair-interleaved layout [128, 4, 2, 1024]; each aggregation
    matmul contracts 256 nodes per instruction (MatmulPerfMode.DoubleRow)
    — 4 MMs per 512-col PSUM tile instead of 8. Halves both PE time and
    adjacency DMA bytes. msg tiles are written in fp8e4 directly by the
    DVE bias-add that drains the linear-layer PSUM.
  - The embedding is folded into layer 0 on the host (W0' = We @ Wl0),
    so the device's first linear runs straight off x (K=64).
  - Diagonal-wavefront schedule over (graph, layer): while a later
    graph's adjacency streams in, earlier graphs' deeper layers keep
    the PE dense; every relu / bias-add latency is covered by another
    graph's aggregation. relu runs full-width on scalar (layers 0/1,
    keeping the DVE queue short for the msg bias-adds) and split
    scalar/DVE for layer 2 (where the projections wait on it).
  - HAM clock management: full-K=128 warm-up matmuls (K=1 rank-1s do
    NOT count as PE activity!) plus a bridge burst hold the 2.4GHz
    un-throttle from ~13us through the last matmul.
  - Projection: 8 chunk matmuls + one rank-1 bias matmul accumulate in
    one PSUM bank; ONE DVE multiply (stride-0 broadcast mask AP) drains
    it to fp16; one contiguous 1KB-per-partition store per graph (out
    DRAM is partition-major; host un-permutes), tail stores dual-ring.
  - DMA: first a small pack with graph 0's lin0 deps, then the
    adjacency as 0.5MB contiguous-per-partition half-graph DMAs, all on
    the SP HWDGE ring (FIFO => smalls can't be starved); everything
    else rides the ACT HWDGE ring in parallel; SWDGE unused.

Scales (exact powers of two, folded into host-side weights):
  adj8 = fp8(16*adj); msg_dev = msg_true/Sm[i]; h_dev = h_true/Sh[i];
  out = fp16((h3@Wp + bp)/So) * mask;  host returns out*So as f32.
"""

import sys
from contextlib import ExitStack

import numpy as np
import ml_dtypes

try:
    import concourse.bass as bass
except ImportError:  # fall back to the repo checkout
    sys.path.insert(0, "/opt/trn_rl_repo")
    import concourse.bass as bass

import concourse.tile as tile
from concourse import bacc, mybir
from concourse.bass_utils import run_bass_kernel_spmd

B, N, IN_DIM, HID, OUT = 32, 1024, 64, 128, 64
NUM_LAYERS = 3
N_CORES = 8
BPC = B // N_CORES  # graphs per core
NC8 = N // 128      # node chunks of 128
NPAIR = NC8 // 2    # DoubleRow chunk pairs (256 nodes each)

FP8 = mybir.dt.float8e4
FP16 = mybir.dt.float16
FP32 = mybir.dt.float32
RELU = mybir.ActivationFunctionType.Relu
COPY = mybir.ActivationFunctionType.Copy
DR = mybir.MatmulPerfMode.DoubleRow

# numeric scales (see module docstring); all exact powers of two
ADJ_SCALE = 16.0
SM = [2.0 ** -5, 2.0 ** -1, 2.0 ** 7]        # msg_dev = msg_true / SM[i]
SH = [None, 2.0 ** -7, 2.0, 2.0 ** 8]        # h_dev = h_true / SH[i]
SO = 2.0 ** 7                                 # out_dev = out_true / SO
RELU_SCALE = [SM[i] / (ADJ_SCALE * SH[i + 1]) for i in range(3)]

def _kernel_body(ctx, tc, out, adj8, xT, maskT, wpack, rows):
    nc = tc.nc

    consts = ctx.enter_context(tc.tile_pool(name="consts", bufs=1))
    adj_pool = ctx.enter_context(tc.tile_pool(name="adj", bufs=BPC))
    xt_pool = ctx.enter_context(tc.tile_pool(name="xt", bufs=BPC))
    mask_pool = ctx.enter_context(tc.tile_pool(name="mask", bufs=BPC))
    h_pool = ctx.enter_context(tc.tile_pool(name="h", bufs=8))
    msg_pool = ctx.enter_context(tc.tile_pool(name="msg", bufs=6))
    o_pool = ctx.enter_context(tc.tile_pool(name="o", bufs=BPC))
    bl_pool = ctx.enter_context(tc.tile_pool(name="bl", bufs=NUM_LAYERS))
    psA = ctx.enter_context(tc.tile_pool(name="psA", bufs=2, space="PSUM"))
    psM = ctx.enter_context(tc.tile_pool(name="psM", bufs=2, space="PSUM"))
    psO = ctx.enter_context(tc.tile_pool(name="psO", bufs=2, space="PSUM"))

    ones_t = consts.tile([1, HID], FP16, tag="ones")
    warm_t = consts.tile([128, 512], FP16, tag="warm")
    nc.vector.memset(ones_t[:], 1.0)
    # full-width warm operand: K=1 rank-1 matmuls do NOT register as PE
    # activity for the HAM clock gate — warm-up must drive all 128 rows.
    nc.vector.memset(warm_t[:], 0.0)

    # wpack layout: cols 0:128 = W0' and 128:1152 = x0 (partitions 0:64
    # only — loaded as the FIRST, small DMA so graph 0's layer-0 linear
    # unblocks ~2us earlier); cols 1152:1472 = Wl1'|Wl2'|Wp' (full 128
    # partitions, second DMA — not needed until layer 1).
    wpack_t = consts.tile([128, 1472], FP16, tag="wpack")
    rows_t = consts.tile([1, 2048], FP16, tag="rows")
    w0_ap = wpack_t[0:IN_DIM, 0:HID]
    wl_ap = [None, wpack_t[:, 1152:1280], wpack_t[:, 1280:1408]]
    wp_ap = wpack_t[:, 1408:1472]
    bp_row = rows_t[:, 1536:2048]

    # SP HWDGE ring, FIFO: compute-critical small loads FIRST so the
    # adjacency flood cannot starve them at the SDMA level; then the
    # adjacency (one 1MB DMA per graph, contiguous 8KB per partition);
    # the output stores are emitted later on this same ring.
    xts = [wpack_t[0:IN_DIM, 128:1152]]
    for bb in range(1, BPC):
        xts.append(xt_pool.tile([IN_DIM, N], FP16, tag="xt", name=f"xt{bb}"))
    nc.sync.dma_start(wpack_t[0:IN_DIM, 0:1152], wpack[0:IN_DIM, 0:1152])
    nc.sync.dma_start(wpack_t[:, 1152:1472], wpack[:, 1152:1472])
    adjs = []
    for bb in range(BPC):
        a = adj_pool.tile([128, NPAIR, 2, N], FP8, tag="adj", name=f"adj{bb}")
        # two half-loads per graph (contiguous 4KB per partition each):
        # the first aggregation's c2=0,1 matmuls release half a transfer
        # earlier, and the tail half still arrives in time.
        nc.sync.dma_start(a[:, 0:2, :, :], adj8[bb, :, 0:2, :, :])
        nc.sync.dma_start(a[:, 2:4, :, :], adj8[bb, :, 2:4, :, :])
        adjs.append(a)

    # ACT HWDGE ring (parallel to SP): bl0's broadcast + the remaining x
    # tiles + the rest of the small tensors. No SWDGE at all — the gpsimd
    # queue spin-up and end-of-kernel drain stay out of the span.
    bl_bcast = []
    bt0 = bl_pool.tile([128, 512], FP16, tag="bl", name="bl0")
    nc.scalar.dma_start(bt0[:], rows[:, 0:512].to_broadcast([128, 512]))
    bl_bcast.append(bt0)
    for bb in range(1, BPC):
        nc.scalar.dma_start(xts[bb][:], xT[bb])
    nc.scalar.dma_start(rows_t[:], rows[:, :])
    for i in range(1, NUM_LAYERS):
        bt = bl_pool.tile([128, 512], FP16, tag="bl", name=f"bl{i}")
        nc.scalar.dma_start(
            bt[:], rows[:, i * 512:(i + 1) * 512].to_broadcast([128, 512]))
        bl_bcast.append(bt)
    masks = []
    for bb in range(BPC):
        m = mask_pool.tile([128, NC8], FP32, tag="mask", name=f"mask{bb}")
        nc.scalar.dma_start(m[:], maskT[bb])
        masks.append(m)

    # HAM warm-up: ~4.8us of dependency-free full-array (K=128) matmuls
    # keep the PE activity window open from ~7us so the 2.4GHz
    # un-throttle fires (~3.4us of sustained busy) before real work.
    warm_ps = psO.tile([128, 512], FP32, tag="psO", name="warm")
    for _ in range(13):
        nc.tensor.matmul(warm_ps[:], warm_t[:, 0:128], warm_t[:],
                         start=True, stop=True)

    def emit_warm_bridge(n):
        # keep the PE activity window open across the wait for the first
        # adjacency bytes — a >2us idle here re-throttles the HAM clock
        # and the whole layer-0 phase then runs at 1.2GHz.
        wps = psA.tile([128, N], FP32, tag="psA", name="warm2")
        for _ in range(n):
            nc.tensor.matmul(wps[:, 0:512], warm_t[:, 0:128],
                             warm_t[:, 0:512], start=True, stop=True)

    def emit_linear_mms(bb, i, h):
        # msg[n, k] = (h @ Wl'[i] + bl'[i]) -> fp8 pair layout
        msg_t = msg_pool.tile([128, NPAIR, 2, HID], FP8, tag="msg",
                              name=f"msg{bb}_{i}")
        pms = []
        for half in range(2):
            pm = psM.tile([128, 512], FP32, tag="psM")
            for k in range(4):
                c = 4 * half + k
                if i == 0:
                    lhsT = xts[bb][:, c * 128:(c + 1) * 128]
                    rhs = w0_ap
                else:
                    lhsT = h[:, c * 128:(c + 1) * 128]
                    rhs = wl_ap[i]
                nc.tensor.matmul(pm[:, k * 128:(k + 1) * 128], lhsT, rhs,
                                 start=True, stop=True)
            pms.append(pm)
        return msg_t, pms

    def emit_linear_drain(msg_t, pms, i):
        for half in range(2):
            half_ap = msg_t[:, 2 * half:2 * half + 2, :, :]
            nc.vector.tensor_add(
                half_ap.rearrange("p a b c -> p (a b c)"), pms[half][:],
                bl_bcast[i][:])

    def emit_linear(bb, i, h):
        msg_t, pms = emit_linear_mms(bb, i, h)
        emit_linear_drain(msg_t, pms, i)
        return msg_t

    def emit_agg(bb, i, msg_t):
        pa = psA.tile([128, N], FP32, tag="psA")
        # c2-major: both t-tiles' matmuls share one weight load per chunk
        # pair; the two banks' accumulation groups interleave.
        for c2 in range(NPAIR):
            for t in range(2):
                nc.tensor.matmul(
                    pa[:, t * 512:(t + 1) * 512],
                    msg_t[:, c2, :, :],
                    adjs[bb][:, c2, :, t * 512:(t + 1) * 512],
                    start=(c2 == 0), stop=(c2 == NPAIR - 1), perf_mode=DR,
                    skip_group_check=True)
        return pa

    def emit_relu(bb, i, pa):
        # Layers 0/1: one full-width ACT on scalar — keeps the relu off
        # the DVE, whose queue (msg bias-adds) is the critical chain into
        # the next aggregation. Layer 2: split scalar/DVE — the DVE has
        # no adds left, and the projection matmuls wait on this relu with
        # only a short wavefront cover.
        h2 = h_pool.tile([HID, N], FP16, tag="h", name=f"h{bb}_{i}")
        if i < NUM_LAYERS - 1:
            nc.scalar.activation(h2[:], pa[:], RELU, scale=RELU_SCALE[i])
        else:
            nc.scalar.activation(h2[:, 0:512], pa[:, 0:512], RELU,
                                 scale=RELU_SCALE[i])
            nc.vector.tensor_scalar(h2[:, 512:1024], pa[:, 512:1024],
                                    RELU_SCALE[i], 0.0,
                                    op0=mybir.AluOpType.mult,
                                    op1=mybir.AluOpType.max)
        return h2

    def emit_proj_mms(bb, h):
        po = psO.tile([128, 512], FP32, tag="psO", name=f"psO{bb}")
        for c in range(NC8):
            nc.tensor.matmul(po[:, c * OUT:(c + 1) * OUT],
                             h[:, c * 128:(c + 1) * 128], wp_ap,
                             start=(c == 0), stop=False, skip_group_check=True)
        nc.tensor.matmul(po[:], ones_t[:], bp_row, start=False, stop=True,
                         skip_group_check=True)
        return po

    def emit_proj_drain(bb, po):
        # single DVE multiply drains the whole projection PSUM with the
        # node mask applied via a stride-0 broadcast AP; one contiguous
        # 1KB-per-partition store follows (out DRAM is partition-major
        # [128, NC8, OUT]; the host un-permutes). Store triggers
        # alternate between the two HWDGE rings.
        o_big = o_pool.tile([128, NC8, OUT], FP16, tag="o", name=f"o{bb}")
        mask_b = masks[bb][:, :].unsqueeze(-1).broadcast_to([128, NC8, OUT])
        nc.vector.tensor_tensor(
            o_big[:], po[:].rearrange("p (a b) -> p a b", a=NC8), mask_b,
            op=mybir.AluOpType.mult)
        if bb >= BPC - 2:
            # tail graphs: halves on both HWDGE rings — the two store
            # triggers and completion receipts overlap.
            nc.sync.dma_start(out[bb][:, 0:4, :], o_big[:, 0:4, :])
            nc.scalar.dma_start(out[bb][:, 4:NC8, :], o_big[:, 4:NC8, :])
        elif bb % 2:
            nc.scalar.dma_start(out[bb], o_big[:])
        else:
            nc.sync.dma_start(out[bb], o_big[:])

    # ---- prologue: warm-up already queued; layer-0 linears + biases ----
    msgs = [None] * BPC
    hs = [None] * BPC
    msgs[0] = emit_linear(0, 0, None)
    for bb in range(1, BPC):
        msgs[bb] = emit_linear(bb, 0, None)
    emit_warm_bridge(10)

    # ---- main: diagonal-wavefront schedule ----
    # Events (aggregations; projection = layer NUM_LAYERS) run in
    # diagonal order over (graph, layer): while a later graph's
    # adjacency is still streaming in, earlier graphs' deeper layers
    # keep the PE dense, so the HAM clock never re-throttles and the
    # tail collapses to the last two projections. Each linear's matmuls
    # are emitted ~2 events before the aggregation that consumes them
    # (covering the relu + DVE bias-add latency); each projection's
    # drain is emitted 2 events after its matmuls.
    events = []
    for d in range(BPC + NUM_LAYERS + 1):
        for g in reversed(range(BPC)):
            i = d - g
            if 0 <= i <= NUM_LAYERS:
                events.append((g, i))
    pos = {ev: k for k, ev in enumerate(events)}
    lin_slot = {}
    for g in range(BPC):
        for i in range(1, NUM_LAYERS):
            lin_slot.setdefault(
                max(pos[(g, i - 1)] + 1, pos[(g, i)] - 2), []).append((g, i))
    drain_slot = {}
    for g in range(BPC):
        drain_slot.setdefault(
            min(pos[(g, NUM_LAYERS)] + 1, len(events)), []).append(g)

    pos_tiles = [None] * BPC
    for k, (g, i) in enumerate(events):
        for (lg, li) in lin_slot.get(k, ()):
            msgs[lg] = emit_linear(lg, li, hs[lg])
        if i < NUM_LAYERS:
            pa = emit_agg(g, i, msgs[g])
            hs[g] = emit_relu(g, i, pa)
        else:
            pos_tiles[g] = emit_proj_mms(g, hs[g])
        for dg in drain_slot.get(k + 1, ()):
            emit_proj_drain(dg, pos_tiles[dg])
    for dg in drain_slot.get(len(events), ()):
        emit_proj_drain(dg, pos_tiles[dg])


def build_nc():
    # Bacc (not raw Bass): its compile() runs generate_event_semaphores,
    # which splits multi-sem waits down to the 1-wait-per-instruction
    # hardware limit walrus enforces.
    nc = bacc.Bacc("TRN2", debug=False, num_devices=N_CORES, num_swdge_queues=2)
    adj8 = nc.dram_tensor("adj8", [BPC, 128, NPAIR, 2, N], FP8,
                          kind="ExternalInput").ap()
    xT = nc.dram_tensor("xT", [BPC, IN_DIM, N], FP16, kind="ExternalInput").ap()
    maskT = nc.dram_tensor("maskT", [BPC, 128, NC8], FP32, kind="ExternalInput").ap()
    wpack = nc.dram_tensor("wpack", [128, 1472], FP16, kind="ExternalInput").ap()
    rows = nc.dram_tensor("rows", [1, 2048], FP16, kind="ExternalInput").ap()
    out = nc.dram_tensor("out", [BPC, 128, NC8, OUT], FP16,
                         kind="ExternalOutput").ap()

    with tile.TileContext(nc) as tc, ExitStack() as ctx:
        _kernel_body(ctx, tc, out, adj8, xT, maskT, wpack, rows)
    nc.compile()
    return nc


def make_in_maps(node_features, adjacency_matrix, node_mask, W_embed, Wl, bl,
                 W_proj, b_proj):
    e4 = ml_dtypes.float8_e4m3
    x = np.asarray(node_features, dtype=np.float32)
    adj = np.asarray(adjacency_matrix, dtype=np.float32)
    mask = np.asarray(node_mask, dtype=np.float32)
    We = np.asarray(W_embed, np.float64)
    Wl64 = np.asarray(Wl, np.float64)
    bl64 = np.asarray(bl, np.float64)
    Wp = np.asarray(W_proj, np.float64)
    bp = np.asarray(b_proj, np.float64)

    wpack = np.zeros((128, 1472), np.float16)
    wpack[:IN_DIM, 0:128] = (We @ Wl64[0] / SM[0]).astype(np.float16)
    wpack[:, 1152:1280] = (Wl64[1] * (SH[1] / SM[1])).astype(np.float16)
    wpack[:, 1280:1408] = (Wl64[2] * (SH[2] / SM[2])).astype(np.float16)
    wpack[:, 1408:1472] = (Wp * (SH[3] / SO)).astype(np.float16)

    rows = np.concatenate(
        [np.tile(bl64[i] / SM[i], 4) for i in range(NUM_LAYERS)]
        + [np.tile(bp / SO, NC8)]).astype(np.float16).reshape(1, 2048)

    in_maps = []
    for cc in range(N_CORES):
        sl = slice(cc * BPC, (cc + 1) * BPC)
        # adj8[bb, j, c2, o, n] = fp8(16 * adj[n, c2*256 + o*128 + j])
        a = np.ascontiguousarray(adj[sl].transpose(0, 2, 1))  # [BPC, m, n]
        a = a.reshape(BPC, NPAIR, 2, 128, N).transpose(0, 3, 1, 2, 4)
        wp_core = wpack.copy()
        wp_core[:IN_DIM, 128:1152] = x[cc * BPC].T.astype(np.float16)
        in_maps.append({
            "adj8": (np.float32(ADJ_SCALE) * a).astype(e4),
            "xT": np.ascontiguousarray(x[sl].transpose(0, 2, 1)).astype(np.float16),
            "maskT": np.ascontiguousarray(
                mask[sl].reshape(BPC, NC8, 128).transpose(0, 2, 1)),
            "wpack": wp_core,
            "rows": rows,
        })
    return in_maps


_NC_CACHE = None


def get_nc():
    global _NC_CACHE
    if _NC_CACHE is None:
        _NC_CACHE = build_nc()
    return _NC_CACHE


def postprocess(raw_out):
    # device layout [BPC, 128(p), NC8(c), OUT] -> natural [BPC, N, OUT]
    # where n = c*128 + p; then undo the output scale.
    o = np.asarray(raw_out, np.float32).transpose(0, 2, 1, 3)
    return o.reshape(BPC, N, OUT) * np.float32(SO)


def kernel(**inputs):
    nc = get_nc()
    in_maps = make_in_maps(**inputs)
    res = run_bass_kernel_spmd(nc, in_maps, list(range(N_CORES)))
    outs = [postprocess(res.results[c]["out"]) for c in range(N_CORES)]
    return np.concatenate(outs, axis=0)


if __name__ == "__main__":
    rng = np.random.default_rng(0)
    ins = {
        "node_features": rng.standard_normal((B, N, IN_DIM), dtype=np.float32),
        "adjacency_matrix": rng.random((B, N, N), dtype=np.float32),
        "node_mask": np.ones((B, N, 1), np.float32),
        "W_embed": rng.standard_normal((IN_DIM, HID), dtype=np.float32) * 0.1,
        "Wl": rng.standard_normal((NUM_LAYERS, HID, HID), dtype=np.float32) * 0.08,
        "bl": rng.standard_normal((NUM_LAYERS, HID), dtype=np.float32) * 0.08,
        "W_proj": rng.standard_normal((HID, 2 * 32), dtype=np.float32) * 0.08,
        "b_proj": rng.standard_normal((2 * 32,), dtype=np.float32) * 0.08,
    }
    out = kernel(**ins)
    print("out", out.shape, out.dtype, float(np.abs(out).mean()))

